# revision 2
# baseline (speedup 1.0000x reference)
"""LIF spike-train kernel for Trainium2 (Bass/Tile), data-parallel over 8 cores.

Reference semantics (T=4, tau=0.5, thresh=1.0), per element:
    mem = 0
    for t in range(4):
        mem = mem*0.5 + x[t]
        s[t] = (mem - 1 >= 0)
        mem = mem - s[t]

x: [T*B, C, H, W] = [256, 128, 32, 32] f32, viewed as [4, 64, 128, 1024].
Batch dim (64) is sharded 8-ways; each core streams [4, 8, 128, 1024].

Every step is bit-exact vs the reference in fp32: mult by 0.5/-1 is exact,
the compare (mem >= 1) <=> (mem - 1 >= 0), and the adds round identically
regardless of fusion.
"""

import os
import sys

sys.path.insert(0, "/opt/trn_rl_repo")

import numpy as np

T = 4
B = 64
C = 128
HW = 1024
NCORES = 8
BLOC = B // NCORES  # 8 batch elements per core

LAST_EXEC_NS = None
LAST_TRACE = None

_CACHE = {}


def _build(bloc=BLOC):
    """Build the per-core Bass module: x [T*bloc, 128, 1024] -> y same shape."""
    import concourse.bacc as bacc
    import concourse.mybir as mybir
    from concourse import tile

    f32 = mybir.dt.float32
    mult = mybir.AluOpType.mult
    add = mybir.AluOpType.add
    is_ge = mybir.AluOpType.is_ge

    nc = bacc.Bacc("TRN2", target_bir_lowering=False, debug=False, num_devices=NCORES)
    x = nc.dram_tensor("x", [T * bloc, C, HW], f32, kind="ExternalInput").ap()
    y = nc.dram_tensor("y", [T * bloc, C, HW], f32, kind="ExternalOutput").ap()

    with tile.TileContext(nc) as tc:
        with tc.tile_pool(name="p", bufs=3) as pool:
            for b in range(bloc):
                xs = []
                for t in range(T):
                    xt = pool.tile([C, HW], f32, tag=f"x{t}")
                    nc.sync.dma_start(out=xt, in_=x[t * bloc + b])
                    xs.append(xt)

                # t = 0: mem = x0; s0 = (mem >= 1); v0 = mem - s0
                s0 = pool.tile([C, HW], f32, tag="s0")
                nc.vector.tensor_scalar(s0, xs[0], 1.0, None, is_ge)
                v = pool.tile([C, HW], f32, tag="v", bufs=4)
                nc.gpsimd.tensor_sub(v, xs[0], s0)
                ss = [s0]

                for t in range(1, T):
                    # u = 0.5*v + x[t]
                    u = pool.tile([C, HW], f32, tag="u", bufs=4)
                    nc.vector.scalar_tensor_tensor(u, v, 0.5, xs[t], mult, add)
                    # s = (u >= 1)
                    st = pool.tile([C, HW], f32, tag=f"s{t}")
                    nc.vector.tensor_scalar(st, u, 1.0, None, is_ge)
                    ss.append(st)
                    if t < T - 1:
                        # v = u - s  (next membrane, post soft-reset)
                        v = pool.tile([C, HW], f32, tag="v", bufs=4)
                        nc.gpsimd.tensor_sub(v, u, ss[t])

                for t in range(T):
                    nc.scalar.dma_start(out=y[t * bloc + b], in_=ss[t])

    nc.compile()
    return nc


def _get_nc():
    if "nc" not in _CACHE:
        _CACHE["nc"] = _build()
    return _CACHE["nc"]


def kernel(x: np.ndarray) -> np.ndarray:
    global LAST_EXEC_NS, LAST_TRACE
    from concourse.bass_utils import run_bass_kernel_spmd

    x = np.ascontiguousarray(np.asarray(x), dtype=np.float32)
    assert x.shape == (T * B, C, 32, 32), x.shape
    xv = x.reshape(T, B, C, HW)

    in_maps = []
    for m in range(NCORES):
        shard = np.ascontiguousarray(xv[:, m * BLOC : (m + 1) * BLOC]).reshape(
            T * BLOC, C, HW
        )
        in_maps.append({"x": shard})

    nc = _get_nc()
    trace = os.environ.get("LIF_TRACE") == "1"
    res = run_bass_kernel_spmd(nc, in_maps, core_ids=list(range(NCORES)), trace=trace)
    LAST_EXEC_NS = res.exec_time_ns
    if res.instructions_and_trace is not None:
        LAST_TRACE = res.instructions_and_trace[1]

    out = np.empty((T, B, C, HW), dtype=np.float32)
    for m in range(NCORES):
        out[:, m * BLOC : (m + 1) * BLOC] = res.results[m]["y"].reshape(
            T, BLOC, C, HW
        )
    return out.reshape(T * B, C, 32, 32)


# revision 3
# speedup vs baseline: 1.0279x; 1.0279x over previous
"""LIF spike-train kernel for Trainium2 (Bass/Tile), data-parallel over 8 cores.

Reference semantics (T=4, tau=0.5, thresh=1.0), per element:
    mem = 0
    for t in range(4):
        mem = mem*0.5 + x[t]
        s[t] = (mem - 1 >= 0)
        mem = mem - s[t]

x: [T*B, C, H, W] = [256, 128, 32, 32] f32, viewed as [4, 64, 128, 1024].
Batch dim (64) is sharded 8-ways; each core streams [4, 8, 128, 1024].

Every step is bit-exact vs the reference in fp32: mult by 0.5/-1 is exact,
the compare (mem >= 1) <=> (mem - 1 >= 0), and the adds round identically
regardless of fusion.
"""

import os
import sys

sys.path.insert(0, "/opt/trn_rl_repo")

import numpy as np

T = 4
B = 64
C = 128
HW = 1024
NCORES = 8
BLOC = B // NCORES  # 8 batch elements per core

LAST_EXEC_NS = None
LAST_TRACE = None

_CACHE = {}


def _build(bloc=BLOC):
    """Build the per-core Bass module: x [T*bloc, 128, 1024] -> y same shape."""
    import concourse.bacc as bacc
    import concourse.mybir as mybir
    from concourse import tile

    f32 = mybir.dt.float32
    mult = mybir.AluOpType.mult
    add = mybir.AluOpType.add
    is_ge = mybir.AluOpType.is_ge

    nc = bacc.Bacc("TRN2", target_bir_lowering=False, debug=False, num_devices=NCORES)
    x = nc.dram_tensor("x", [T * bloc, C, HW], f32, kind="ExternalInput").ap()
    y = nc.dram_tensor("y", [T * bloc, C, HW], f32, kind="ExternalOutput").ap()

    # Stage-major emission over groups of G batches: engines execute their
    # instruction streams in order, so batch-by-batch emission serializes the
    # per-batch dependency chain across engines. Interleaving G independent
    # batch chains at each stage keeps DVE/GpSimd/DMA all busy.
    G = 2
    with tile.TileContext(nc) as tc:
        with tc.tile_pool(name="p", bufs=4) as pool:
            for g in range(0, bloc, G):
                bs = list(range(g, min(g + G, bloc)))
                xs = {}
                for t in range(T):
                    for b in bs:
                        xt = pool.tile([C, HW], f32, tag=f"x{t}")
                        nc.sync.dma_start(out=xt, in_=x[t * bloc + b])
                        xs[t, b] = xt

                # t = 0: mem = x0; s0 = (mem >= 1); v0 = mem - s0
                s = {}
                for b in bs:
                    s0 = pool.tile([C, HW], f32, tag="s0")
                    nc.vector.tensor_scalar(s0, xs[0, b], 1.0, None, is_ge)
                    s[0, b] = s0
                vs = {}
                for b in bs:
                    v = pool.tile([C, HW], f32, tag="v")
                    nc.gpsimd.tensor_sub(v, xs[0, b], s[0, b])
                    vs[b] = v
                for b in bs:
                    nc.scalar.dma_start(out=y[0 * bloc + b], in_=s[0, b])

                for t in range(1, T):
                    us = {}
                    for b in bs:
                        # u = 0.5*v + x[t]
                        u = pool.tile([C, HW], f32, tag="u")
                        nc.vector.scalar_tensor_tensor(
                            u, vs[b], 0.5, xs[t, b], mult, add
                        )
                        us[b] = u
                    for b in bs:
                        # s = (u >= 1)
                        st = pool.tile([C, HW], f32, tag=f"s{t}")
                        nc.vector.tensor_scalar(st, us[b], 1.0, None, is_ge)
                        s[t, b] = st
                    if t < T - 1:
                        for b in bs:
                            # v = u - s  (next membrane, post soft-reset)
                            v = pool.tile([C, HW], f32, tag="v")
                            nc.gpsimd.tensor_sub(v, us[b], s[t, b])
                            vs[b] = v
                    for b in bs:
                        nc.scalar.dma_start(out=y[t * bloc + b], in_=s[t, b])

    nc.compile()
    return nc


def _get_nc():
    if "nc" not in _CACHE:
        _CACHE["nc"] = _build()
    return _CACHE["nc"]


def kernel(x: np.ndarray) -> np.ndarray:
    global LAST_EXEC_NS, LAST_TRACE
    from concourse.bass_utils import run_bass_kernel_spmd

    x = np.ascontiguousarray(np.asarray(x), dtype=np.float32)
    assert x.shape == (T * B, C, 32, 32), x.shape
    xv = x.reshape(T, B, C, HW)

    in_maps = []
    for m in range(NCORES):
        shard = np.ascontiguousarray(xv[:, m * BLOC : (m + 1) * BLOC]).reshape(
            T * BLOC, C, HW
        )
        in_maps.append({"x": shard})

    nc = _get_nc()
    trace = os.environ.get("LIF_TRACE") == "1"
    res = run_bass_kernel_spmd(nc, in_maps, core_ids=list(range(NCORES)), trace=trace)
    LAST_EXEC_NS = res.exec_time_ns
    if res.instructions_and_trace is not None:
        LAST_TRACE = res.instructions_and_trace[1]

    out = np.empty((T, B, C, HW), dtype=np.float32)
    for m in range(NCORES):
        out[:, m * BLOC : (m + 1) * BLOC] = res.results[m]["y"].reshape(
            T, BLOC, C, HW
        )
    return out.reshape(T * B, C, 32, 32)


# revision 6
# speedup vs baseline: 1.0415x; 1.0132x over previous
"""LIF spike-train kernel for Trainium2 (Bass/Tile), data-parallel over 8 cores.

Reference semantics (T=4, tau=0.5, thresh=1.0), per element:
    mem = 0
    for t in range(4):
        mem = mem*0.5 + x[t]
        s[t] = (mem - 1 >= 0)
        mem = mem - s[t]

x: [T*B, C, H, W] = [256, 128, 32, 32] f32, viewed as [4, 64, 128, 1024].
Batch dim (64) is sharded 8-ways; each core streams [4, 8, 128, 1024].

Every step is bit-exact vs the reference in fp32: mult by 0.5/-1 is exact,
the compare (mem >= 1) <=> (mem - 1 >= 0), and the adds round identically
regardless of fusion.
"""

import os
import sys

sys.path.insert(0, "/opt/trn_rl_repo")

import numpy as np

T = 4
B = 64
C = 128
HW = 1024
NCORES = 8
BLOC = B // NCORES  # 8 batch elements per core

LAST_EXEC_NS = None
LAST_TRACE = None

_CACHE = {}


def _build(bloc=BLOC):
    """Build the per-core Bass module: x [T*bloc, 128, 1024] -> y same shape."""
    import concourse.bacc as bacc
    import concourse.mybir as mybir
    from concourse import tile

    f32 = mybir.dt.float32
    mult = mybir.AluOpType.mult
    add = mybir.AluOpType.add
    is_ge = mybir.AluOpType.is_ge

    nc = bacc.Bacc("TRN2", target_bir_lowering=False, debug=False, num_devices=NCORES)
    x = nc.dram_tensor("x", [T * bloc, C, HW], f32, kind="ExternalInput").ap()
    y = nc.dram_tensor("y", [T * bloc, C, HW], f32, kind="ExternalOutput").ap()

    # Stage-major emission over groups of G batches: engines execute their
    # instruction streams in order, so batch-by-batch emission serializes the
    # per-batch dependency chain across engines. Interleaving G independent
    # batch chains at each stage keeps DVE/GpSimd/DMA all busy.
    G = int(os.environ.get("LIF_G", "4"))
    with tile.TileContext(nc) as tc:
        with tc.tile_pool(name="p", bufs=int(os.environ.get("LIF_BUFS", "4"))) as pool:
            for g in range(0, bloc, G):
                bs = list(range(g, min(g + G, bloc)))
                xs = {}
                for t in range(T):
                    for b in bs:
                        xt = pool.tile([C, HW], f32, tag=f"x{t}")
                        nc.sync.dma_start(out=xt, in_=x[t * bloc + b])
                        xs[t, b] = xt

                # t = 0: mem = x0; s0 = (mem >= 1); v0 = mem - s0
                s = {}
                for b in bs:
                    s0 = pool.tile([C, HW], f32, tag="s0")
                    nc.vector.tensor_scalar(s0, xs[0, b], 1.0, None, is_ge)
                    s[0, b] = s0
                vs = {}
                for b in bs:
                    v = pool.tile([C, HW], f32, tag="v", bufs=6)
                    nc.gpsimd.tensor_sub(v, xs[0, b], s[0, b])
                    vs[b] = v
                for b in bs:
                    nc.scalar.dma_start(out=y[0 * bloc + b], in_=s[0, b])

                for t in range(1, T):
                    us = {}
                    for b in bs:
                        # u = 0.5*v + x[t]
                        u = pool.tile([C, HW], f32, tag="u", bufs=6)
                        nc.vector.scalar_tensor_tensor(
                            u, vs[b], 0.5, xs[t, b], mult, add
                        )
                        us[b] = u
                    for b in bs:
                        # s = (u >= 1)
                        st = pool.tile([C, HW], f32, tag=f"s{t}")
                        nc.vector.tensor_scalar(st, us[b], 1.0, None, is_ge)
                        s[t, b] = st
                    if t < T - 1:
                        for b in bs:
                            # v = u - s  (next membrane, post soft-reset)
                            v = pool.tile([C, HW], f32, tag="v", bufs=6)
                            nc.gpsimd.tensor_sub(v, us[b], s[t, b])
                            vs[b] = v
                    for b in bs:
                        nc.scalar.dma_start(out=y[t * bloc + b], in_=s[t, b])

    nc.compile()
    return nc


def _get_nc():
    if "nc" not in _CACHE:
        _CACHE["nc"] = _build()
    return _CACHE["nc"]


def kernel(x: np.ndarray) -> np.ndarray:
    global LAST_EXEC_NS, LAST_TRACE
    from concourse.bass_utils import run_bass_kernel_spmd

    x = np.ascontiguousarray(np.asarray(x), dtype=np.float32)
    assert x.shape == (T * B, C, 32, 32), x.shape
    xv = x.reshape(T, B, C, HW)

    in_maps = []
    for m in range(NCORES):
        shard = np.ascontiguousarray(xv[:, m * BLOC : (m + 1) * BLOC]).reshape(
            T * BLOC, C, HW
        )
        in_maps.append({"x": shard})

    nc = _get_nc()
    trace = os.environ.get("LIF_TRACE") == "1"
    res = run_bass_kernel_spmd(nc, in_maps, core_ids=list(range(NCORES)), trace=trace)
    LAST_EXEC_NS = res.exec_time_ns
    if res.instructions_and_trace is not None:
        LAST_TRACE = res.instructions_and_trace[1]

    out = np.empty((T, B, C, HW), dtype=np.float32)
    for m in range(NCORES):
        out[:, m * BLOC : (m + 1) * BLOC] = res.results[m]["y"].reshape(
            T, BLOC, C, HW
        )
    return out.reshape(T * B, C, 32, 32)
